# revision 1
# baseline (speedup 1.0000x reference)
"""Trainium2 Bass kernel for block-diagonal (chunked) causal self-attention.

Reference computation (per nn.Module):
    qkv = x @ w_attn.T; q,k,v = split(qkv)
    per (batch, head, chunk of 256 tokens): causal softmax attention within chunk
    out = y @ w_proj.T

Sharding: the 16384 tokens (B*T) are split contiguously across 8 cores
(2048 tokens = 8 chunks per core; chunks never cross a core boundary, and
attention is chunk-local, so no collectives are needed).

On-chip layout (per core), all matmul operands bf16, fp32 accumulation:
  xT   [1024, 2048]  x-shard transposed (feature-major)
  qkT = wqkT.T @ xT  [2048, tok]  (q rows 0:1024, k rows 1024:2048)
  v    [tok, 1024]   natural layout, stored with a ones column per head
                     ([128, 16, 65] tiles) so the PV matmul also produces
                     the softmax denominator (unnormalized softmax trick:
                     exp without max-subtraction is safe; scores ~ +-2).
  S^T  [256k, 256q] per (chunk, head) in PSUM = kT.T @ qT
  PT   = exp(0.125*S^T) * causal_mask   (bf16)
  psum_y [65, 256] = [v|1].T @ PT  -> rows 0:64 y^T unnormalized, row 64 = denom
  yT   [64, tok] per head (all partition-base-0 so DVE never shifts lanes)
  out  = sum_h yT_h.T @ wpT_h  [tok, 1024] fp32

Processing is pipelined over 4 phases of 512 tokens (2 chunks) with
double-buffered phase tiles.
"""
import sys
import os

if '/opt/trn_rl_repo' not in sys.path:
    sys.path.insert(0, '/opt/trn_rl_repo')

import numpy as np
import ml_dtypes

import concourse.bass as bass
import concourse.mybir as mybir
import concourse.tile as tile
from concourse.bass_utils import run_bass_kernel_spmd

# problem shape (hardcoded per spec)
B, T, D, H, CS = 4, 4096, 1024, 16, 256
DH = D // H            # 64
NCORES = 8
TOK = (B * T) // NCORES   # 2048 tokens per core
PH_TOK = 512              # tokens per phase
NPH = TOK // PH_TOK       # 4 phases
CPP = PH_TOK // CS        # 2 chunks per phase
MT = PH_TOK // 128        # 4 token tiles per phase
KD = D // 128             # 8 feature k-tiles

F32 = mybir.dt.float32
BF16 = mybir.dt.bfloat16
EXP = mybir.ActivationFunctionType.Exp


def _split_excess_waits(nc, max_waits=1):
    """This container's walrus accepts at most one sync-wait per instruction;
    the Tile tail drain is emitted post-legalize with one wait per live proc.
    Hoist excess waits onto standalone EventSemaphore instructions."""
    for f in nc.m.functions:
        for bb in f.blocks:
            new_insts = []
            for ins in bb.instructions:
                si = ins.sync_info
                waits = list(si.on_wait) if si is not None and si.on_wait else []
                if len(waits) > max_waits:
                    for i, w in enumerate(waits[:-max_waits]):
                        ev = mybir.InstEventSemaphore(
                            name=f"{ins.name}_wsplit{i}", engine=ins.engine,
                            ins=[], outs=[],
                            sync_info=mybir.SyncInfo(on_wait=[w], on_update=[]))
                        new_insts.append(ev)
                    si.on_wait = waits[-max_waits:]
                new_insts.append(ins)
            bb.instructions = new_insts


def _build_nc():
    nc = bass.Bass()
    xT = nc.declare_dram_parameter("xT", [D, TOK], BF16, isOutput=False)
    wqkT = nc.declare_dram_parameter("wqkT", [D, 2 * D], BF16, isOutput=False)
    wvT = nc.declare_dram_parameter("wvT", [D, D], BF16, isOutput=False)
    wpT = nc.declare_dram_parameter("wpT", [D, D], BF16, isOutput=False)
    masks = nc.declare_dram_parameter("masks", [2, 128, CS], BF16, isOutput=False)
    out = nc.declare_dram_parameter("out", [TOK, D], F32, isOutput=True)

    with tile.TileContext(nc) as tc:
        with tc.tile_pool(name="wpool", bufs=1) as wpool, \
             tc.tile_pool(name="ph", bufs=2) as ph, \
             tc.tile_pool(name="wk", bufs=4) as wk, \
             tc.tile_pool(name="dr", bufs=8, space="DRAM") as dr, \
             tc.tile_pool(name="pmm", bufs=2, space="PSUM") as pmm, \
             tc.tile_pool(name="pst", bufs=4, space="PSUM") as pst, \
             tc.tile_pool(name="py", bufs=2, space="PSUM") as py:

            # ---- static weights ----
            wqk_sb = []
            for k in range(KD):
                t = wpool.tile([128, 2 * D], BF16, name=f"wqk{k}")
                nc.sync.dma_start(out=t, in_=wqkT[k * 128:(k + 1) * 128, :])
                wqk_sb.append(t)
            wv_sb = []
            for k in range(KD):
                t = wpool.tile([128, D], BF16, name=f"wv{k}")
                nc.sync.dma_start(out=t, in_=wvT[k * 128:(k + 1) * 128, :])
                wv_sb.append(t)
            wp_sb = []
            for h in range(H):
                t = wpool.tile([64, D], BF16, name=f"wp{h}")
                nc.sync.dma_start(out=t, in_=wpT[h * 64:(h + 1) * 64, :])
                wp_sb.append(t)
            m0 = wpool.tile([128, CS], BF16, name="mask0")
            nc.sync.dma_start(out=m0, in_=masks[0])
            m1 = wpool.tile([128, CS], BF16, name="mask1")
            nc.sync.dma_start(out=m1, in_=masks[1])

            for p in range(NPH):
                tok0 = p * PH_TOK
                # ---- load x^T slice ----
                xk = []
                for k in range(KD):
                    t = ph.tile([128, PH_TOK], BF16, name=f"xk{k}", tag=f"xk{k}")
                    nc.sync.dma_start(
                        out=t, in_=xT[k * 128:(k + 1) * 128, tok0:tok0 + PH_TOK])
                    xk.append(t)

                # ---- qk projection: qkT[f, t] (f-tile major) ----
                qk_sb = []
                for f in range(2 * KD):
                    ps_ = pmm.tile([128, PH_TOK], F32, name="psmm", tag="mm")
                    for k in range(KD):
                        nc.tensor.matmul(
                            ps_, wqk_sb[k][:, f * 128:(f + 1) * 128], xk[k],
                            start=(k == 0), stop=(k == KD - 1))
                    t = ph.tile([128, PH_TOK], BF16, name=f"qk{f}", tag=f"qk{f}")
                    nc.scalar.copy(out=t, in_=ps_)
                    qk_sb.append(t)

                # ---- v projection (natural layout + ones column per head) ----
                vp_sb = []
                for m in range(MT):
                    t = ph.tile([128, H, DH + 1], BF16, name=f"vp{m}", tag=f"vp{m}")
                    for n2 in range(2):
                        ps_ = pmm.tile([128, 512], F32, name="psmm", tag="mm")
                        for k in range(KD):
                            nc.tensor.matmul(
                                ps_, xk[k][:, m * 128:(m + 1) * 128],
                                wv_sb[k][:, n2 * 512:(n2 + 1) * 512],
                                start=(k == 0), stop=(k == KD - 1))
                        nc.vector.tensor_copy(
                            out=t[:, n2 * 8:(n2 + 1) * 8, 0:DH],
                            in_=ps_.rearrange("p (h d) -> p h d", d=DH))
                    nc.gpsimd.memset(t[:, :, DH:DH + 1], 1.0)
                    vp_sb.append(t)

                # ---- per-head yT tiles (partition base 0) ----
                yT_sb = [ph.tile([64, PH_TOK], BF16, name=f"yT{h}", tag=f"yT{h}")
                         for h in range(H)]

                # ---- block-diagonal causal attention ----
                for c in range(CPP):
                    col0 = c * CS
                    for h in range(H):
                        ft, rh = h // 2, (h % 2) * 64
                        qT = qk_sb[ft][rh:rh + 64, col0:col0 + CS]
                        kT = qk_sb[KD + ft][rh:rh + 64, col0:col0 + CS]
                        pts = []
                        for kk in range(2):
                            st = pst.tile([128, CS], F32, name="psst", tag="st")
                            nc.tensor.matmul(
                                st, kT[:, kk * 128:(kk + 1) * 128], qT,
                                start=True, stop=True)
                            pt = wk.tile([128, CS], BF16, name="pt", tag="pt",
                                         bufs=6)
                            nc.scalar.activation(out=pt, in_=st, func=EXP,
                                                 scale=0.125)
                            nc.vector.tensor_mul(pt, pt, m0 if kk == 0 else m1)
                            pts.append(pt)
                        ps_y = py.tile([DH + 1, CS], F32, name="psy", tag="y")
                        for kk in range(2):
                            vsl = vp_sb[CPP * c + kk][:, h, :]
                            nc.tensor.matmul(ps_y, vsl, pts[kk],
                                             start=(kk == 0), stop=(kk == 1))
                        linv = wk.tile([1, CS], F32, name="linv", tag="linv")
                        nc.vector.reciprocal(out=linv, in_=ps_y[DH:DH + 1, :])
                        lscr = dr.tile([1, CS], F32, name="lscr", tag="lscr")
                        nc.sync.dma_start(out=lscr, in_=linv)
                        rrep = wk.tile([64, CS], F32, name="rrep", tag="rrep")
                        bc = bass.AP(tensor=lscr.tensor, offset=lscr.offset,
                                     ap=[[0, 64]] + [list(q) for q in lscr.ap[1:]])
                        nc.sync.dma_start(out=rrep, in_=bc)
                        nc.vector.tensor_mul(
                            yT_sb[h][:, col0:col0 + CS], ps_y[0:DH, :], rrep)

                # ---- output projection ----
                for m in range(MT):
                    for n in range(2):
                        ps_ = pmm.tile([128, 512], F32, name="psmm", tag="mm")
                        for h in range(H):
                            nc.tensor.matmul(
                                ps_, yT_sb[h][:, m * 128:(m + 1) * 128],
                                wp_sb[h][:, n * 512:(n + 1) * 512],
                                start=(h == 0), stop=(h == H - 1))
                        ost = wk.tile([128, 512], F32, name="ost", tag="ost",
                                      bufs=3)
                        nc.vector.tensor_copy(out=ost, in_=ps_)
                        nc.sync.dma_start(
                            out=out[tok0 + m * 128: tok0 + (m + 1) * 128,
                                    n * 512:(n + 1) * 512],
                            in_=ost)

    _split_excess_waits(nc)
    return nc


_NC_CACHE = None


def _get_nc():
    global _NC_CACHE
    if _NC_CACHE is None:
        _NC_CACHE = _build_nc()
    return _NC_CACHE


def _prep_shared(w_attn, w_proj):
    wqkT = np.ascontiguousarray(w_attn[:2 * D, :].T).astype(ml_dtypes.bfloat16)
    wvT = np.ascontiguousarray(w_attn[2 * D:, :].T).astype(ml_dtypes.bfloat16)
    wpT = np.ascontiguousarray(w_proj.T).astype(ml_dtypes.bfloat16)
    ii = np.arange(128)[:, None]
    qq = np.arange(CS)[None, :]
    masks = np.stack([(ii <= qq), (ii + 128 <= qq)]).astype(ml_dtypes.bfloat16)
    return wqkT, wvT, wpT, masks


def kernel(x, w_attn, w_proj, _trace=False):
    x = np.asarray(x)
    w_attn = np.asarray(w_attn)
    w_proj = np.asarray(w_proj)
    wqkT, wvT, wpT, masks = _prep_shared(w_attn, w_proj)
    x_flat = x.reshape(B * T, D)
    in_maps = []
    for c in range(NCORES):
        xTc = np.ascontiguousarray(
            x_flat[c * TOK:(c + 1) * TOK, :].T).astype(ml_dtypes.bfloat16)
        in_maps.append({"xT": xTc, "wqkT": wqkT, "wvT": wvT, "wpT": wpT,
                        "masks": masks})
    nc = _get_nc()
    kw = {}
    if _trace:
        kw["trace"] = True
    res = run_bass_kernel_spmd(nc, in_maps, core_ids=list(range(NCORES)), **kw)
    outs = [res.results[c]["out"] for c in range(NCORES)]
    full = np.concatenate(outs, axis=0).reshape(B, T, D)
    if _trace:
        return full, res
    return full


# revision 2
# speedup vs baseline: 1.2616x; 1.2616x over previous
"""Trainium2 Bass kernel for block-diagonal (chunked) causal self-attention.

Reference computation (per nn.Module):
    qkv = x @ w_attn.T; q,k,v = split(qkv)
    per (batch, head, chunk of 256 tokens): causal softmax attention in-chunk
    out = y @ w_proj.T

Sharding: the 16384 tokens (B*T) are split contiguously across 8 cores
(2048 tokens = 8 chunks per core; chunks never cross a core boundary and
attention is chunk-local, so no collectives are needed).

Per-core on-chip dataflow (matmul operands bf16, fp32 accumulation):
  xT   [1024, 2048]   x-shard transposed (feature-major)
  qkT  = wqkT.T @ xT  [2048, tok] (q rows 0:1024, k rows 1024:2048)
  v    [tok, 1024]    natural layout with a ones column per head
                      ([128, 16, 65] tiles) so the PV matmul also produces
                      softmax denominators (exp without max-subtraction is
                      safe: scores ~ +-2)
  S^T  [256k, 256q]   per (chunk, head), both k-tiles in ONE [128, 512]
                      PSUM tile = kT.T @ qT
  PT   = exp(0.125*S^T) * causal_mask   (one ACT exp + one DVE mul, bf16)
  psum_y [65, 256] = [v|1].T @ PT  (rows 0:64 y^T unnorm, row 64 = denom l)
  linv = exp(-ln(l)) on ACT (DVE reciprocal is 7 cyc/elem - way too slow),
         broadcast across partitions via a DRAM bounce (engines are
         partition-locked; DMA is the only cheap lane shuffle)
  yT   [64, tok] per head (all partition-base-0 so DVE never lane-shifts)
  out  = sum_h yT_h.T @ wpT_h  [tok, 1024] fp32

4 phases of 512 tokens, double-buffered. The attention inner loop is
software-pipelined (S^T matmuls run PIPE_DEPTH blocks ahead of the PV
matmuls) so the in-order PE stream never waits on the exp/mask chain.
"""
import sys

if '/opt/trn_rl_repo' not in sys.path:
    sys.path.insert(0, '/opt/trn_rl_repo')

import numpy as np
import ml_dtypes

import concourse.bass as bass
import concourse.mybir as mybir
import concourse.tile as tile
from concourse.bass_utils import run_bass_kernel_spmd

# problem shape (hardcoded per spec)
B, T, D, H, CS = 4, 4096, 1024, 16, 256
DH = D // H            # 64
NCORES = 8
TOK = (B * T) // NCORES   # 2048 tokens per core
PH_TOK = 512              # tokens per phase
NPH = TOK // PH_TOK       # 4 phases
CPP = PH_TOK // CS        # 2 chunks per phase
MT = PH_TOK // 128        # 4 token tiles per phase
KD = D // 128             # 8 feature k-tiles
PIPE = 2                  # attention block software-pipeline depth

F32 = mybir.dt.float32
BF16 = mybir.dt.bfloat16
EXP = mybir.ActivationFunctionType.Exp
LN = mybir.ActivationFunctionType.Ln


def _split_excess_waits(nc, max_waits=1):
    """This container's walrus accepts at most one sync-wait per instruction;
    the Tile tail drain is emitted post-legalize with one wait per live proc.
    Hoist excess waits onto standalone EventSemaphore instructions."""
    for f in nc.m.functions:
        for bb in f.blocks:
            new_insts = []
            for ins in bb.instructions:
                si = ins.sync_info
                waits = list(si.on_wait) if si is not None and si.on_wait else []
                if len(waits) > max_waits:
                    for i, w in enumerate(waits[:-max_waits]):
                        ev = mybir.InstEventSemaphore(
                            name=f"{ins.name}_wsplit{i}", engine=ins.engine,
                            ins=[], outs=[],
                            sync_info=mybir.SyncInfo(on_wait=[w], on_update=[]))
                        new_insts.append(ev)
                    si.on_wait = waits[-max_waits:]
                new_insts.append(ins)
            bb.instructions = new_insts


def _build_nc():
    nc = bass.Bass()
    xT = nc.declare_dram_parameter("xT", [D, TOK], BF16, isOutput=False)
    wqkT = nc.declare_dram_parameter("wqkT", [D, 2 * D], BF16, isOutput=False)
    wvT = nc.declare_dram_parameter("wvT", [D, D], BF16, isOutput=False)
    wpT = nc.declare_dram_parameter("wpT", [D, D], BF16, isOutput=False)
    masks = nc.declare_dram_parameter("masks", [128, 2 * CS], BF16, isOutput=False)
    out = nc.declare_dram_parameter("out", [TOK, D], F32, isOutput=True)

    with tile.TileContext(nc) as tc:
        with tc.tile_pool(name="wpool", bufs=1) as wpool, \
             tc.tile_pool(name="ph", bufs=2) as ph, \
             tc.tile_pool(name="wk", bufs=4) as wk, \
             tc.tile_pool(name="dr", bufs=12, space="DRAM") as dr, \
             tc.tile_pool(name="pmm", bufs=2, space="PSUM") as pmm, \
             tc.tile_pool(name="pst", bufs=3, space="PSUM") as pst, \
             tc.tile_pool(name="py", bufs=3, space="PSUM") as py:

            # ---- static weights ----
            wqk_sb = []
            for k in range(KD):
                t = wpool.tile([128, 2 * D], BF16, name=f"wqk{k}")
                nc.sync.dma_start(out=t, in_=wqkT[k * 128:(k + 1) * 128, :])
                wqk_sb.append(t)
            wv_sb = []
            for k in range(KD):
                t = wpool.tile([128, D], BF16, name=f"wv{k}")
                nc.sync.dma_start(out=t, in_=wvT[k * 128:(k + 1) * 128, :])
                wv_sb.append(t)
            wp_sb = []
            for h in range(H):
                t = wpool.tile([64, D], BF16, name=f"wp{h}")
                nc.sync.dma_start(out=t, in_=wpT[h * 64:(h + 1) * 64, :])
                wp_sb.append(t)
            msk = wpool.tile([128, 2 * CS], BF16, name="msk")
            nc.sync.dma_start(out=msk, in_=masks[:, :])

            for p in range(NPH):
                tok0 = p * PH_TOK
                # ---- load x^T slice ----
                xk = []
                for k in range(KD):
                    t = ph.tile([128, PH_TOK], BF16, name=f"xk{k}", tag=f"xk{k}")
                    nc.sync.dma_start(
                        out=t, in_=xT[k * 128:(k + 1) * 128, tok0:tok0 + PH_TOK])
                    xk.append(t)

                # ---- qk projection: qkT[f, t] (f-tile major) ----
                qk_sb = []
                for f in range(2 * KD):
                    ps_ = pmm.tile([128, PH_TOK], F32, name="psmm", tag="mm")
                    for k in range(KD):
                        nc.tensor.matmul(
                            ps_, wqk_sb[k][:, f * 128:(f + 1) * 128], xk[k],
                            start=(k == 0), stop=(k == KD - 1))
                    t = ph.tile([128, PH_TOK], BF16, name=f"qk{f}", tag=f"qk{f}")
                    nc.vector.tensor_copy(out=t, in_=ps_)
                    qk_sb.append(t)

                # ---- v projection (natural layout + ones column per head) ----
                vp_sb = []
                for m in range(MT):
                    t = ph.tile([128, H, DH + 1], BF16, name=f"vp{m}", tag=f"vp{m}")
                    for n2 in range(2):
                        ps_ = pmm.tile([128, 512], F32, name="psmm", tag="mm")
                        for k in range(KD):
                            nc.tensor.matmul(
                                ps_, xk[k][:, m * 128:(m + 1) * 128],
                                wv_sb[k][:, n2 * 512:(n2 + 1) * 512],
                                start=(k == 0), stop=(k == KD - 1))
                        nc.vector.tensor_copy(
                            out=t[:, n2 * 8:(n2 + 1) * 8, 0:DH],
                            in_=ps_.rearrange("p (h d) -> p h d", d=DH))
                    nc.gpsimd.memset(t[:, :, DH:DH + 1], 1.0)
                    vp_sb.append(t)

                # ---- per-head yT tiles (partition base 0) ----
                yT_sb = [ph.tile([64, PH_TOK], BF16, name=f"yT{h}", tag=f"yT{h}")
                         for h in range(H)]

                # ---- attention: software-pipelined over 32 blocks ----
                def stage1(c, h):
                    """S^T matmuls (both k-tiles into one [128,512] psum),
                    exp, causal mask -> PT tile."""
                    col0 = c * CS
                    ft, rh = h // 2, (h % 2) * 64
                    qT = qk_sb[ft][rh:rh + 64, col0:col0 + CS]
                    kT = qk_sb[KD + ft][rh:rh + 64, col0:col0 + CS]
                    st = pst.tile([128, 2 * CS], F32, name="psst", tag="st")
                    for kk in range(2):
                        nc.tensor.matmul(
                            st[:, kk * CS:(kk + 1) * CS],
                            kT[:, kk * 128:(kk + 1) * 128], qT,
                            start=True, stop=True)
                    pt = wk.tile([128, 2 * CS], BF16, name="pt", tag="pt", bufs=6)
                    nc.scalar.activation(out=pt, in_=st, func=EXP, scale=0.125)
                    nc.vector.tensor_mul(pt, pt, msk)
                    return pt

                def stage2(c, h, pt):
                    """PV matmul (+ones row), linv = exp(-ln(l)) on ACT,
                    partition-broadcast via DRAM bounce, normalize into yT."""
                    col0 = c * CS
                    ps_y = py.tile([DH + 1, CS], F32, name="psy", tag="y")
                    for kk in range(2):
                        vsl = vp_sb[CPP * c + kk][:, h, :]
                        nc.tensor.matmul(ps_y, vsl, pt[:, kk * CS:(kk + 1) * CS],
                                         start=(kk == 0), stop=(kk == 1))
                    lnl = wk.tile([1, CS], F32, name="lnl", tag="lnl", bufs=6)
                    nc.scalar.activation(out=lnl, in_=ps_y[DH:DH + 1, :], func=LN)
                    linv = wk.tile([1, CS], F32, name="linv", tag="linv", bufs=6)
                    nc.scalar.activation(out=linv, in_=lnl, func=EXP, scale=-1.0)
                    lscr = dr.tile([1, CS], F32, name="lscr", tag="lscr")
                    nc.sync.dma_start(out=lscr, in_=linv)
                    rrep = wk.tile([64, CS], F32, name="rrep", tag="rrep", bufs=6)
                    bc = bass.AP(tensor=lscr.tensor, offset=lscr.offset,
                                 ap=[[0, 64]] + [list(q) for q in lscr.ap[1:]])
                    nc.sync.dma_start(out=rrep, in_=bc)
                    nc.vector.tensor_mul(
                        yT_sb[h][:, col0:col0 + CS], ps_y[0:DH, :], rrep)

                def out_proj(m_lo, m_hi):
                    for m in range(m_lo, m_hi):
                        for n in range(2):
                            ps_ = pmm.tile([128, 512], F32, name="psmm", tag="mm")
                            for h in range(H):
                                nc.tensor.matmul(
                                    ps_, yT_sb[h][:, m * 128:(m + 1) * 128],
                                    wp_sb[h][:, n * 512:(n + 1) * 512],
                                    start=(h == 0), stop=(h == H - 1))
                            ost = wk.tile([128, 512], F32, name="ost", tag="ost",
                                          bufs=3)
                            nc.vector.tensor_copy(out=ost, in_=ps_)
                            nc.sync.dma_start(
                                out=out[tok0 + m * 128: tok0 + (m + 1) * 128,
                                        n * 512:(n + 1) * 512],
                                in_=ost)

                blocks = [(c, h) for c in range(CPP) for h in range(H)]
                pending = []  # [(c, h, pt), ...] stage1 done, stage2 not yet
                done_s2 = 0
                for bi, (c, h) in enumerate(blocks):
                    pt = stage1(c, h)
                    pending.append((c, h, pt))
                    if len(pending) > PIPE:
                        c2, h2, pt2 = pending.pop(0)
                        stage2(c2, h2, pt2)
                        done_s2 += 1
                        if done_s2 == H:       # chunk 0 fully normalized
                            out_proj(0, 2)
                for c2, h2, pt2 in pending:
                    stage2(c2, h2, pt2)
                out_proj(2, MT)

    _split_excess_waits(nc)
    return nc


_NC_CACHE = None


def _get_nc():
    global _NC_CACHE
    if _NC_CACHE is None:
        _NC_CACHE = _build_nc()
    return _NC_CACHE


def _prep_shared(w_attn, w_proj):
    wqkT = np.ascontiguousarray(w_attn[:2 * D, :].T).astype(ml_dtypes.bfloat16)
    wvT = np.ascontiguousarray(w_attn[2 * D:, :].T).astype(ml_dtypes.bfloat16)
    wpT = np.ascontiguousarray(w_proj.T).astype(ml_dtypes.bfloat16)
    ii = np.arange(128)[:, None]
    qq = np.arange(CS)[None, :]
    masks = np.concatenate([(ii <= qq), (ii + 128 <= qq)],
                           axis=1).astype(ml_dtypes.bfloat16)   # [128, 512]
    return wqkT, wvT, wpT, masks


def kernel(x, w_attn, w_proj, _trace=False):
    x = np.asarray(x)
    w_attn = np.asarray(w_attn)
    w_proj = np.asarray(w_proj)
    wqkT, wvT, wpT, masks = _prep_shared(w_attn, w_proj)
    x_flat = x.reshape(B * T, D)
    in_maps = []
    for c in range(NCORES):
        xTc = np.ascontiguousarray(
            x_flat[c * TOK:(c + 1) * TOK, :].T).astype(ml_dtypes.bfloat16)
        in_maps.append({"xT": xTc, "wqkT": wqkT, "wvT": wvT, "wpT": wpT,
                        "masks": masks})
    nc = _get_nc()
    kw = {}
    if _trace:
        kw["trace"] = True
    res = run_bass_kernel_spmd(nc, in_maps, core_ids=list(range(NCORES)), **kw)
    outs = [res.results[c]["out"] for c in range(NCORES)]
    full = np.concatenate(outs, axis=0).reshape(B, T, D)
    if _trace:
        return full, res
    return full
